# revision 12
# baseline (speedup 1.0000x reference)
"""Trainium2 Bass kernel for nn_MultiHeadAttention_66872640799208.

Math (per batch element b, S=2048, D=1024):
    qp = q @ Wq.T + bq ; kp = k @ Wk.T + bk ; vp = v @ Wv.T + bv
    scores = qp @ kp.T / D
    probs  = softmax(scores, axis=q)          # over the QUERY axis
    attn   = probs @ vp
    attn_w = softmax(attn, axis=q)            # over the sequence axis
    out    = (attn + q, attn_w)

Sharding: data-parallel over batch B=8 -> one batch element per NeuronCore,
no collectives.

All five big matmuls run in fp8e4m3 with MatmulPerfMode.DoubleRow: each
instruction contracts K=256 (two 128-partition tiles packed along the free
dim of both operands) at 0.5 cycles per output row -- 4x bf16 MACs/cycle in
the cost model. Layouts (host pre-transposes, casts to fp8):
  - qT/kT/vT [D, S] fp8 feed the projections (contraction over d),
  - qpT/kpT are produced in [e, s] fp8 so scoresT = kpT.T @ qpT has the
    softmax axis (q) on the free dimension,
  - vp is produced in natural [s, e] bf16, then quantized to fp8 with the
    softmax-1 normalization folded in as (2048/Z_k) ~= 1.0 (fp8-safe; the
    leftover global 1/2048 moves into the exp scale of softmax #2 and a
    host-side divide of the residual output -- both exact).
The attn matmul psum therefore carries 2048*attn; the residual add uses
host-prescaled 2048*q (bf16) and the host divides the output by 2048.
The second softmax (over q, the partition axis) uses a ones-vector PE
matmul for column sums (bf16 expb -- fp8 there would put its quantization
noise directly on attn_w) and a K=1 PE matmul to broadcast 1/Z.

SBUF per partition: wall 24K / probs 32K (tag W), qp 16K / expb 32K
(tag A), kp 16K / aw 8K (tag B), vp16 32K, vp8 16K, io ~20K: ~150KB.
"""

import sys

if "/opt/trn_rl_repo" not in sys.path:
    sys.path.insert(0, "/opt/trn_rl_repo")

import numpy as np
import ml_dtypes

B = 8
S = 2048
D = 1024
P = 128


def build_nc(s=S, d=D):
    """Build the single-core Bass program (SPMD: identical on all cores)."""
    import concourse.bass as bass
    import concourse.tile as tile
    from concourse import bacc, mybir

    bf16 = mybir.dt.bfloat16
    fp8 = mybir.dt.float8e4
    f32 = mybir.dt.float32
    DR = mybir.MatmulPerfMode.DoubleRow

    DT = d // P          # contraction tiles for projections
    DH = DT // 2         # DoubleRow pairs over d
    ET = d // P          # e (feature) tiles
    EH = ET // 2         # DoubleRow pairs over e
    ST = s // P          # sequence tiles
    SH = ST // 2         # DoubleRow pairs over s (attn contraction)
    NFQ = min(512, s)    # matmul moving free-dim over q
    QC = s // NFQ        # q chunks
    NFD = min(512, d)    # matmul moving free-dim over d/e
    EC = d // NFD        # e chunks
    SCW = min(512, s)    # vT stream chunk width (in s)
    SC = s // SCW

    nc = bacc.Bacc("TRN2")

    qT = nc.dram_tensor("qT", [d, s], fp8, kind="ExternalInput")
    kT = nc.dram_tensor("kT", [d, s], fp8, kind="ExternalInput")
    vT = nc.dram_tensor("vT", [d, s], fp8, kind="ExternalInput")
    wqT = nc.dram_tensor("wqT", [d, d], fp8, kind="ExternalInput")  # [d, e]
    wkT = nc.dram_tensor("wkT", [d, d], fp8, kind="ExternalInput")
    wvT = nc.dram_tensor("wvT", [d, d], fp8, kind="ExternalInput")
    bq = nc.dram_tensor("bq", [d], f32, kind="ExternalInput")
    bk = nc.dram_tensor("bk", [d], f32, kind="ExternalInput")
    bv = nc.dram_tensor("bv", [d], f32, kind="ExternalInput")
    qres = nc.dram_tensor("qres", [s, d], bf16, kind="ExternalInput")  # 2048*q
    attn_o = nc.dram_tensor("attn", [s, d], bf16, kind="ExternalOutput")
    attnw_o = nc.dram_tensor("attn_w", [s, d], bf16, kind="ExternalOutput")

    qT_r = qT[:].rearrange("(dt p) s -> p dt s", p=P)
    kT_r = kT[:].rearrange("(dt p) s -> p dt s", p=P)
    vT_r = vT[:].rearrange("(dt p) s -> p dt s", p=P)
    w_rs = [
        w[:].rearrange("(dt p) e -> p dt e", p=P) for w in (wqT, wkT, wvT)
    ]
    bq_r = bq[:].rearrange("(t p) -> p t", p=P)
    bk_r = bk[:].rearrange("(t p) -> p t", p=P)
    qres_r = qres[:].rearrange("(st p) d -> p st d", p=P)
    attn_r = attn_o[:].rearrange("(st p) d -> p st d", p=P)
    attnw_r = attnw_o[:].rearrange("(st p) d -> p st d", p=P)

    with tile.TileContext(nc) as tc:
        with (
            tc.tile_pool(name="consts", bufs=1) as consts,
            tc.tile_pool(name="big", bufs=1) as big,
            tc.tile_pool(name="io", bufs=2) as io,
            tc.tile_pool(name="small", bufs=4) as small,
            tc.tile_pool(name="psum", bufs=4, space="PSUM") as psum,
            tc.tile_pool(name="psum1", bufs=1, space="PSUM") as psum1,
        ):
            # ---- constants. Weights/biases ride the gpsimd SWDGE queue so
            # the sync queue is free for the qT/kT/vT stream and the scalar
            # engine's instruction stream stays pure activations: the PE's
            # first matmul only waits on wall0 + the first q-chunk. ----
            wall = big.tile([P, 3, DT, d], fp8, tag="W")   # wq|wk|wv
            nc.gpsimd.dma_start(out=wall[:, 0], in_=w_rs[0])
            bq_t = consts.tile([P, ET], f32)
            nc.gpsimd.dma_start(out=bq_t[:], in_=bq_r)
            bk_t = consts.tile([P, ET], f32)
            bv_bc = consts.tile([P, d], f32)
            ones_col = consts.tile([P, 1], bf16)   # lhsT for column sums (K=P, M=1)
            nc.vector.memset(ones_col[:], 1.0)
            rz_all = consts.tile([P, ST], f32)     # per-k-row 1/Z of softmax #1
            rz2 = consts.tile([1, d], f32)         # 1/colsum of softmax #2
            rzb_sb = consts.tile([P, d], bf16)     # bcast 1/colsum, bf16
            ones_row = consts.tile([1, P], f32)    # lhsT for 1/Z broadcast (K=1)
            nc.vector.memset(ones_row[:], 1.0)

            qp = big.tile([P, ET, s], fp8, tag="A")         # qpT: [e, q]
            kp = big.tile([P, ET, s], fp8, tag="B")         # kpT: [e, k]
            vp16 = big.tile([P, ST, d], bf16, tag="V2")     # natural [s, e]
            vp8 = big.tile([P, ST, d], fp8, tag="V")        # *(2048/Z_k)

            # ---- Phase 1a: qpT / kpT projections ----
            for src_r, wi, b_t, dst in (
                (qT_r, 0, bq_t, qp),
                (kT_r, 1, bk_t, kp),
            ):
                for qc in range(QC):
                    xt = io.tile([P, DT, NFQ], fp8, tag="xin")
                    nc.sync.dma_start(
                        out=xt[:], in_=src_r[:, :, qc * NFQ:(qc + 1) * NFQ]
                    )
                    if wi == 0 and qc == 0:
                        # remaining weights stream on their own (gpsimd) queue
                        nc.gpsimd.dma_start(out=wall[:, 1], in_=w_rs[1])
                        nc.gpsimd.dma_start(out=wall[:, 2], in_=w_rs[2])
                        nc.gpsimd.dma_start(out=bk_t[:], in_=bk_r)
                        bv_ap = bv[:]
                        nc.gpsimd.dma_start(
                            out=bv_bc[:],
                            in_=bass.AP(
                                tensor=bv_ap.tensor,
                                offset=bv_ap.offset,
                                ap=[[0, P], [1, d]],
                            ),
                        )
                    for et in range(ET):
                        ps = psum.tile([P, NFQ], f32, tag="ps")
                        for j in range(DH):
                            nc.tensor.matmul(
                                ps[:],
                                wall[:, wi, 2 * j:2 * j + 2, et * P:(et + 1) * P],
                                xt[:, 2 * j:2 * j + 2, :],
                                start=(j == 0),
                                stop=(j == DH - 1),
                                perf_mode=DR,
                            )
                        # bias add (per-partition) + fp8 cast on ScalarE
                        nc.scalar.activation(
                            out=dst[:, et, qc * NFQ:(qc + 1) * NFQ],
                            in_=ps[:],
                            func=mybir.ActivationFunctionType.Identity,
                            bias=b_t[:, et:et + 1],
                        )

            # ---- Phase 1b: vp projection (natural layout, bf16) ----
            for sc in range(SC):
                vt = io.tile([P, DT, SCW], fp8, tag="xin")
                nc.sync.dma_start(
                    out=vt[:], in_=vT_r[:, :, sc * SCW:(sc + 1) * SCW]
                )
                for sti in range(SCW // P):
                    st = sc * (SCW // P) + sti
                    for ec in range(EC):
                        ps = psum.tile([P, NFD], f32, tag="ps")
                        for j in range(DH):
                            nc.tensor.matmul(
                                ps[:],
                                vt[:, 2 * j:2 * j + 2, sti * P:(sti + 1) * P],
                                wall[:, 2, 2 * j:2 * j + 2,
                                     ec * NFD:(ec + 1) * NFD],
                                start=(j == 0),
                                stop=(j == DH - 1),
                                perf_mode=DR,
                            )
                        nc.vector.tensor_add(
                            out=vp16[:, st, ec * NFD:(ec + 1) * NFD],
                            in0=ps[:],
                            in1=bv_bc[:, ec * NFD:(ec + 1) * NFD],
                        )

            # ---- Phase 2: scoresT -> softmax over q -> probs (fp8) ----
            # probs reuses the weights' slot (tag W).
            # No max-subtraction: |scores/d| < ~0.3 by construction.
            probs = big.tile([P, ST, s], fp8, tag="W")      # [k, q] per k-tile
            for kt in range(ST):
                for qc in range(QC):
                    ps = psum.tile([P, NFQ], f32, tag="ps")
                    for j in range(EH):
                        nc.tensor.matmul(
                            ps[:],
                            kp[:, 2 * j:2 * j + 2, kt * P:(kt + 1) * P],
                            qp[:, 2 * j:2 * j + 2, qc * NFQ:(qc + 1) * NFQ],
                            start=(j == 0),
                            stop=(j == EH - 1),
                            perf_mode=DR,
                        )
                    nc.scalar.activation(
                        out=probs[:, kt, qc * NFQ:(qc + 1) * NFQ],
                        in_=ps[:],
                        func=mybir.ActivationFunctionType.Exp,
                        scale=1.0 / d,
                    )
                # Z from the quantized probs on the DVE — keeps the scalar
                # engine off the ACTIVATION_READ_ACCUMULATOR path (the fp8
                # reduce is the slow 1x path but fits under the PE phase)
                zsum = small.tile([P, 1], f32, tag="zsum")
                nc.vector.reduce_sum(
                    out=zsum[:], in_=probs[:, kt, :], axis=mybir.AxisListType.X
                )
                nc.vector.reciprocal(out=rz_all[:, kt:kt + 1], in_=zsum[:])
                # quantize vp to fp8 with (2048/Z_k) folded into its k-rows:
                # ~1.0 so the fp8 range is preserved; the global 1/2048 is
                # repaid at the attn psum (exp scale + host divide).
                nc.vector.tensor_scalar(
                    out=vp8[:, kt, :],
                    in0=vp16[:, kt, :],
                    scalar1=rz_all[:, kt:kt + 1],
                    scalar2=float(s),
                    op0=mybir.AluOpType.mult,
                    op1=mybir.AluOpType.mult,
                )

            # ---- Phase 3: 2048*attn = probsT.T @ vp8 ; residual; exp ----
            # expb reuses qp's slot (tag A).
            expb = big.tile([P, ST, d], bf16, tag="A")      # exp(attn), bf16
            cs_ps = psum1.tile([1, d], f32, tag="cs")       # colsums of exp(attn)
            for st in range(ST):
                qres_t = io.tile([P, d], bf16, tag="xin")
                # gpsimd SWDGE queue: keeps the sync queue free for attn out
                nc.gpsimd.dma_start(out=qres_t[:], in_=qres_r[:, st, :])
                ao = io.tile([P, d], bf16, tag="ao")
                for ec in range(EC):
                    ps = psum.tile([P, NFD], f32, tag="ps")
                    for j in range(SH):
                        nc.tensor.matmul(
                            ps[:],
                            probs[:, 2 * j:2 * j + 2, st * P:(st + 1) * P],
                            vp8[:, 2 * j:2 * j + 2, ec * NFD:(ec + 1) * NFD],
                            start=(j == 0),
                            stop=(j == SH - 1),
                            perf_mode=DR,
                        )
                    nc.vector.tensor_add(
                        out=ao[:, ec * NFD:(ec + 1) * NFD],
                        in0=ps[:],
                        in1=qres_t[:, ec * NFD:(ec + 1) * NFD],
                    )
                    nc.scalar.activation(
                        out=expb[:, st, ec * NFD:(ec + 1) * NFD],
                        in_=ps[:],
                        func=mybir.ActivationFunctionType.Exp,
                        scale=1.0 / s,
                    )
                    nc.tensor.matmul(
                        cs_ps[:, ec * NFD:(ec + 1) * NFD],
                        ones_col[:],
                        expb[:, st, ec * NFD:(ec + 1) * NFD],
                        start=(st == 0),
                        stop=(st == ST - 1),
                    )
                nc.sync.dma_start(out=attn_r[:, st, :], in_=ao[:])

            # ---- Phase 3.5: 1/colsum, broadcast across partitions ----
            # approx recip: ~51 ULP, ~5x faster; Z ~ s +- 5% is edge-case-safe
            nc.vector.reciprocal_approx_fast(out=rz2[:], in_=cs_ps[:])
            rzb = psum1.tile([P, d], f32, tag="cs")         # reuses cs_ps bank
            for ec in range(EC):
                nc.tensor.matmul(
                    rzb[:, ec * NFD:(ec + 1) * NFD],
                    ones_row[:],
                    rz2[:, ec * NFD:(ec + 1) * NFD],
                    start=True,
                    stop=True,
                )
            # bf16 copy so the phase-4 multiplies run at 2x 16-bit DVE rate
            nc.scalar.copy(out=rzb_sb[:], in_=rzb[:])

            # ---- Phase 4: attn_w = exp(attn) * (1/colsum) ----
            # Pure tail (depends on the global colsum): groups of NG=2
            # s-tiles; multiplies alternate vector/gpsimd so the two engines
            # halve the serial mul chain, DMAs alternate the sync/scalar
            # HWDGE queues so 1MB writes overlap. rzb is read through a
            # stride-0 AP to broadcast it across the NG tiles of a group.
            NG = min(2, ST)
            aw_all = big.tile([P, 4, NG, d], bf16, tag="B")
            rz_ap = rzb_sb[:]
            rz_bc = bass.AP(
                tensor=rz_ap.tensor,
                offset=rz_ap.offset,
                ap=[rz_ap.ap[0], [0, NG], [1, d]],
            )
            for g in range(ST // NG):
                aw = aw_all[:, g % 4]
                mul_eng = nc.vector if g % 2 == 0 else nc.gpsimd
                mul_eng.tensor_mul(
                    out=aw, in0=expb[:, g * NG:(g + 1) * NG, :], in1=rz_bc
                )
                dma_eng = nc.sync if g % 2 == 0 else nc.scalar
                dma_eng.dma_start(out=attnw_r[:, g * NG:(g + 1) * NG, :], in_=aw)

    return nc


def _host_prep(q, k, v, Wq, bq, Wk, bk, Wv, bv):
    """Shard over batch and pre-transpose/cast on host."""
    fp8 = ml_dtypes.float8_e4m3
    bf16 = ml_dtypes.bfloat16
    q = np.asarray(q, dtype=np.float32)
    k = np.asarray(k, dtype=np.float32)
    v = np.asarray(v, dtype=np.float32)
    wqT = np.asarray(Wq, dtype=np.float32).T.astype(fp8)  # [d, e]
    wkT = np.asarray(Wk, dtype=np.float32).T.astype(fp8)
    wvT = np.asarray(Wv, dtype=np.float32).T.astype(fp8)
    bq = np.ascontiguousarray(np.asarray(bq, dtype=np.float32))
    bk = np.ascontiguousarray(np.asarray(bk, dtype=np.float32))
    bv = np.ascontiguousarray(np.asarray(bv, dtype=np.float32))

    in_maps = []
    for i in range(B):
        in_maps.append(
            {
                "qT": q[i].T.astype(fp8),
                "kT": k[i].T.astype(fp8),
                "vT": v[i].T.astype(fp8),
                "wqT": wqT,
                "wkT": wkT,
                "wvT": wvT,
                "bq": bq,
                "bk": bk,
                "bv": bv,
                "qres": (q[i] * float(S)).astype(bf16),
            }
        )
    return in_maps


def _host_post(attn_raw, attnw_raw):
    """Undo the 2048x psum scaling and widen to f32."""
    attn = attn_raw.astype(np.float32) * (1.0 / float(S))
    attn_w = attnw_raw.astype(np.float32)
    return attn, attn_w


_CACHED_NC = None


def kernel(q, k, v, Wq, bq, Wk, bk, Wv, bv):
    global _CACHED_NC
    from concourse import bass_utils

    in_maps = _host_prep(q, k, v, Wq, bq, Wk, bk, Wv, bv)
    if _CACHED_NC is None:
        _CACHED_NC = build_nc()
        _CACHED_NC.finalize()  # bacc passes (reg alloc, wait splitting)
    res = bass_utils.run_bass_kernel_spmd(
        _CACHED_NC, in_maps, core_ids=list(range(B))
    )
    attn = np.stack([np.asarray(res.results[i]["attn"]) for i in range(B)])
    attn_w = np.stack([np.asarray(res.results[i]["attn_w"]) for i in range(B)])
    return _host_post(attn, attn_w)


# revision 27
# speedup vs baseline: 1.0358x; 1.0358x over previous
"""Trainium2 Bass kernel for nn_MultiHeadAttention_66872640799208.

Math (per batch element b, S=2048, D=1024):
    qp = q @ Wq.T + bq ; kp = k @ Wk.T + bk ; vp = v @ Wv.T + bv
    scores = qp @ kp.T / D
    probs  = softmax(scores, axis=q)          # over the QUERY axis
    attn   = probs @ vp
    attn_w = softmax(attn, axis=q)            # over the sequence axis
    out    = (attn + q, attn_w)

Sharding: data-parallel over batch B=8 -> one batch element per NeuronCore,
no collectives.

Scores reassociation: qp.kp^T = q (Wq^T Wk) k^T + q.(Wq^T bk) + (bq^T Wk).k
+ bq.bk. The last two terms are constant along the softmax (query) axis and
cancel; the host precomputes M2 = Wq^T Wk and wqbk = Wq^T bk, so the kp
projection disappears entirely (raw kT is the scores lhsT, already in [d, s]
layout) and qp becomes u = q @ M2 (no bias). The alpha_i = q.wqbk term is
added into each scores psum group via a K=1 ones-broadcast matmul.

All big matmuls run in fp8e4m3 with MatmulPerfMode.DoubleRow: each
instruction contracts K=256 (two 128-partition tiles packed along the free
dim of both operands) -- 2x bf16 MACs/cycle on TRN2 hardware. Layouts (host
pre-transposes, casts to fp8):
  - qT/kT/vT [D, S] fp8: qT feeds the u projection + alpha matvec, kT is
    the scores stationary operand, vT feeds the vp projection,
  - u is produced in [b, q] fp8 layout so scoresT = kT.T @ u has the
    softmax axis (q) on the free dimension,
  - vp is produced in natural [s, e] bf16, then quantized to fp8 with the
    softmax-1 normalization folded in as (2048/Z_k) ~= 1.0 (fp8-safe; the
    leftover global 1/2048 moves into the exp scale of softmax #2 and a
    host-side divide of the residual output -- both exact).
The attn matmul psum therefore carries 2048*attn; the residual add uses
host-prescaled 2048*q (bf16) and the host divides the output by 2048.
The second softmax (over q, the partition axis) uses a ones-vector PE
matmul for column sums (bf16 expb -- fp8 there would put its quantization
noise directly on attn_w) and a K=1 PE matmul to broadcast 1/Z.

DMA: everything bulk rides the sync HWDGE queue, with the weight/aux
dispatches interleaved between the qT chunk dispatches so no input chunk
waits behind a 1MB weight transfer. qres rides the gpsimd SWDGE queue
(not latency-critical), outputs ride sync + scalar HWDGE queues.
"""

import sys

if "/opt/trn_rl_repo" not in sys.path:
    sys.path.insert(0, "/opt/trn_rl_repo")

import numpy as np
import ml_dtypes

B = 8
S = 2048
D = 1024
P = 128


def build_nc(s=S, d=D):
    """Build the single-core Bass program (SPMD: identical on all cores)."""
    import concourse.bass as bass
    import concourse.tile as tile
    from concourse import bacc, mybir

    bf16 = mybir.dt.bfloat16
    fp8 = mybir.dt.float8e4
    f32 = mybir.dt.float32
    DR = mybir.MatmulPerfMode.DoubleRow

    DT = d // P          # contraction tiles for projections / scores
    DH = DT // 2         # DoubleRow pairs over d
    ET = d // P          # e (feature) tiles
    ETA = ET + 1         # u tiles incl. the alpha column tile
    ST = s // P          # sequence tiles
    SH = ST // 2         # DoubleRow pairs over s (attn contraction)
    NFQ = min(512, s)    # matmul moving free-dim over q
    QC = s // NFQ        # q chunks
    NFD = min(512, d)    # matmul moving free-dim over d/e
    EC = d // NFD        # e chunks
    SCW = min(512, s)    # vT stream chunk width (in s)
    SC = s // SCW

    nc = bacc.Bacc("TRN2")

    qT = nc.dram_tensor("qT", [d, s], fp8, kind="ExternalInput")
    kT = nc.dram_tensor("kT", [d, s], fp8, kind="ExternalInput")
    vT = nc.dram_tensor("vT", [d, s], fp8, kind="ExternalInput")
    # m2 = [Wq^T Wk | Wq^T bk | 0-pad]: the alpha column rides the u
    # projection as tile ET, partition-row 0
    m2 = nc.dram_tensor("m2", [d, ETA * P], fp8, kind="ExternalInput")
    wvT = nc.dram_tensor("wvT", [d, d], fp8, kind="ExternalInput")
    bv = nc.dram_tensor("bv", [d], f32, kind="ExternalInput")
    qres = nc.dram_tensor("qres", [s, d], bf16, kind="ExternalInput")  # 2048*q
    attn_o = nc.dram_tensor("attn", [s, d], bf16, kind="ExternalOutput")
    attnw_o = nc.dram_tensor("attn_w", [s, d], bf16, kind="ExternalOutput")

    qT_r = qT[:].rearrange("(dt p) s -> p dt s", p=P)
    kT_r = kT[:].rearrange("(dt p) s -> p dt s", p=P)
    vT_r = vT[:].rearrange("(dt p) s -> p dt s", p=P)
    m2_r = m2[:].rearrange("(dt p) e -> p dt e", p=P)
    wv_r = wvT[:].rearrange("(dt p) e -> p dt e", p=P)
    qres_r = qres[:].rearrange("(st p) d -> p st d", p=P)
    attn_r = attn_o[:].rearrange("(st p) d -> p st d", p=P)
    attnw_r = attnw_o[:].rearrange("(st p) d -> p st d", p=P)

    with tile.TileContext(nc) as tc:
        with (
            tc.tile_pool(name="consts", bufs=1) as consts,
            tc.tile_pool(name="big", bufs=1) as big,
            tc.tile_pool(name="io", bufs=2) as io,
            tc.tile_pool(name="small", bufs=4) as small,
            tc.tile_pool(name="psum", bufs=4, space="PSUM") as psum,
            tc.tile_pool(name="psum1", bufs=1, space="PSUM") as psum1,
        ):
            # wall holds m2 (ETA col-tiles) then wv (EC chunks) along free
            wall = big.tile([P, DT, ETA * P + d], fp8, tag="W")
            WV0 = ETA * P      # wv column offset within wall
            bv_bc = consts.tile([P, d], f32)
            ones_col = consts.tile([P, 1], bf16)   # lhsT for column sums
            nc.vector.memset(ones_col[:], 1.0)
            ones_a = consts.tile([1, P], fp8)      # lhsT for alpha broadcast
            nc.vector.memset(ones_a[:], 1.0)
            ones_row = consts.tile([1, P], f32)    # lhsT for 1/Z broadcast
            nc.vector.memset(ones_row[:], 1.0)
            rz_all = consts.tile([P, ST], f32)     # per-k-row 1/Z of softmax 1
            rz2 = consts.tile([1, d], f32)         # 1/colsum of softmax 2
            rzb_sb = consts.tile([P, d], bf16)     # bcast 1/colsum, bf16

            u = big.tile([P, ETA, s], fp8, tag="A")         # uT: [b, q] + alpha
            kT_sb = big.tile([P, DT, s], fp8, tag="B")      # raw kT
            vp16 = big.tile([P, ST, d], bf16, tag="V2")     # natural [s, e]
            vp8 = big.tile([P, ST, d], fp8, tag="V")        # *(2048/Z_k)

            # first weight + first q-chunk dispatch up front (sync queue)
            nc.sync.dma_start(out=wall[:, :, 0:ETA * P], in_=m2_r)

            # ---- Phase 1a: u projection (alpha = col ET, partition-row 0) --
            for qc in range(QC):
                xt = io.tile([P, DT, NFQ], fp8, tag="xin")
                nc.sync.dma_start(
                    out=xt[:], in_=qT_r[:, :, qc * NFQ:(qc + 1) * NFQ]
                )
                # interleave the remaining aux transfers between q-chunk
                # dispatches: nothing latency-critical ever queues behind
                # more than ~1MB
                if qc == min(1, QC - 1):
                    nc.sync.dma_start(out=wall[:, :, WV0:WV0 + d], in_=wv_r)
                if qc == min(2, QC - 1):
                    bv_ap = bv[:]
                    nc.sync.dma_start(
                        out=bv_bc[:],
                        in_=bass.AP(
                            tensor=bv_ap.tensor,
                            offset=bv_ap.offset,
                            ap=[[0, P], [1, d]],
                        ),
                    )
                for et in range(ETA):
                    ps = psum.tile([P, NFQ], f32, tag="ps")
                    for j in range(DH):
                        nc.tensor.matmul(
                            ps[:],
                            wall[:, 2 * j:2 * j + 2, et * P:(et + 1) * P],
                            xt[:, 2 * j:2 * j + 2, :],
                            start=(j == 0),
                            stop=(j == DH - 1),
                            perf_mode=DR,
                        )
                        if et == 0 and j == 0 and qc == QC - 1:
                            # kT for the scores phase: dispatched once the
                            # sync queue has drained the critical aux loads
                            nc.sync.dma_start(out=kT_sb[:], in_=kT_r)
                    nc.scalar.copy(
                        out=u[:, et, qc * NFQ:(qc + 1) * NFQ], in_=ps[:]
                    )

            # ---- Phase 1b: vp projection (natural layout, bf16) ----
            for sc in range(SC):
                vt = io.tile([P, DT, SCW], fp8, tag="xin")
                nc.sync.dma_start(
                    out=vt[:], in_=vT_r[:, :, sc * SCW:(sc + 1) * SCW]
                )
                for sti in range(SCW // P):
                    st = sc * (SCW // P) + sti
                    for ec in range(EC):
                        ps = psum.tile([P, NFD], f32, tag="ps")
                        for j in range(DH):
                            nc.tensor.matmul(
                                ps[:],
                                vt[:, 2 * j:2 * j + 2, sti * P:(sti + 1) * P],
                                wall[:, 2 * j:2 * j + 2,
                                     WV0 + ec * NFD:WV0 + (ec + 1) * NFD],
                                start=(j == 0),
                                stop=(j == DH - 1),
                                perf_mode=DR,
                            )
                        nc.vector.tensor_add(
                            out=vp16[:, st, ec * NFD:(ec + 1) * NFD],
                            in0=ps[:],
                            in1=bv_bc[:, ec * NFD:(ec + 1) * NFD],
                        )

            # ---- Phase 2: scoresT -> softmax over q -> probs (fp8) ----
            # scoresT = kT.T @ u + 1^T alpha; probs reuses the weights' slot.
            # No max-subtraction: |scores/d| < ~0.3 by construction.
            probs = big.tile([P, ST, s], fp8, tag="W")      # [k, q] per k-tile
            for kt in range(ST):
                for qc in range(QC):
                    ps = psum.tile([P, NFQ], f32, tag="ps")
                    for j in range(DH):
                        nc.tensor.matmul(
                            ps[:],
                            kT_sb[:, 2 * j:2 * j + 2, kt * P:(kt + 1) * P],
                            u[:, 2 * j:2 * j + 2, qc * NFQ:(qc + 1) * NFQ],
                            start=(j == 0),
                            stop=False,
                            perf_mode=DR,
                        )
                    nc.tensor.matmul(
                        ps[:],
                        ones_a[:],
                        u[0:1, ET, qc * NFQ:(qc + 1) * NFQ],
                        start=False,
                        stop=True,
                    )
                    nc.scalar.activation(
                        out=probs[:, kt, qc * NFQ:(qc + 1) * NFQ],
                        in_=ps[:],
                        func=mybir.ActivationFunctionType.Exp,
                        scale=1.0 / d,
                    )
                # Z from the quantized probs on the DVE — keeps the scalar
                # engine off the ACTIVATION_READ_ACCUMULATOR path (the fp8
                # reduce is the slow 1x path but fits under the PE phase)
                zsum = small.tile([P, 1], f32, tag="zsum")
                nc.vector.reduce_sum(
                    out=zsum[:], in_=probs[:, kt, :], axis=mybir.AxisListType.X
                )
                nc.vector.reciprocal(out=rz_all[:, kt:kt + 1], in_=zsum[:])
                # quantize vp to fp8 with (2048/Z_k) folded into its k-rows:
                # ~1.0 so the fp8 range is preserved; the global 1/2048 is
                # repaid at the attn psum (exp scale + host divide).
                nc.vector.tensor_scalar(
                    out=vp8[:, kt, :],
                    in0=vp16[:, kt, :],
                    scalar1=rz_all[:, kt:kt + 1],
                    scalar2=float(s),
                    op0=mybir.AluOpType.mult,
                    op1=mybir.AluOpType.mult,
                )

            # ---- Phase 3: 2048*attn = probsT.T @ vp8 ; residual; exp ----
            # expb reuses u's slot (tag A).
            expb = big.tile([P, ST, d], bf16, tag="A")      # exp(attn), bf16
            cs_ps = psum1.tile([1, d], f32, tag="cs")       # colsums of exp
            for st in range(ST):
                qres_t = io.tile([P, d], bf16, tag="xin")
                # gpsimd SWDGE queue: keeps the sync queue free for attn out
                nc.gpsimd.dma_start(out=qres_t[:], in_=qres_r[:, st, :])
                ao = io.tile([P, d], bf16, tag="ao")
                for ec in range(EC):
                    ps = psum.tile([P, NFD], f32, tag="ps")
                    for j in range(SH):
                        nc.tensor.matmul(
                            ps[:],
                            probs[:, 2 * j:2 * j + 2, st * P:(st + 1) * P],
                            vp8[:, 2 * j:2 * j + 2, ec * NFD:(ec + 1) * NFD],
                            start=(j == 0),
                            stop=(j == SH - 1),
                            perf_mode=DR,
                        )
                    nc.vector.tensor_add(
                        out=ao[:, ec * NFD:(ec + 1) * NFD],
                        in0=ps[:],
                        in1=qres_t[:, ec * NFD:(ec + 1) * NFD],
                    )
                    nc.scalar.activation(
                        out=expb[:, st, ec * NFD:(ec + 1) * NFD],
                        in_=ps[:],
                        func=mybir.ActivationFunctionType.Exp,
                        scale=1.0 / s,
                    )
                    nc.tensor.matmul(
                        cs_ps[:, ec * NFD:(ec + 1) * NFD],
                        ones_col[:],
                        expb[:, st, ec * NFD:(ec + 1) * NFD],
                        start=(st == 0),
                        stop=(st == ST - 1),
                    )
                nc.sync.dma_start(out=attn_r[:, st, :], in_=ao[:])

            # ---- Phase 3.5: 1/colsum, broadcast across partitions ----
            # approx recip: ~51 ULP, ~5x faster; Z ~ s +- 5% is edge-case-safe
            nc.vector.reciprocal_approx_fast(out=rz2[:], in_=cs_ps[:])
            rzb = psum1.tile([P, d], f32, tag="cs")         # reuses cs_ps bank
            for ec in range(EC):
                nc.tensor.matmul(
                    rzb[:, ec * NFD:(ec + 1) * NFD],
                    ones_row[:],
                    rz2[:, ec * NFD:(ec + 1) * NFD],
                    start=True,
                    stop=True,
                )
            # bf16 copy so the phase-4 multiplies run at 2x 16-bit DVE rate
            nc.scalar.copy(out=rzb_sb[:], in_=rzb[:])

            # ---- Phase 4: attn_w = exp(attn) * (1/colsum) ----
            # Pure tail (depends on the global colsum): groups of NG=2
            # s-tiles, multiplies on the DVE (gpsimd is 2x slower — measured),
            # DMAs alternating the sync/scalar HWDGE queues so 1MB writes
            # overlap. rzb is read through a stride-0 AP to broadcast it
            # across the NG tiles of a group.
            NG = min(2, ST)
            aw_all = big.tile([P, 4, NG, d], bf16, tag="B")
            rz_ap = rzb_sb[:]
            rz_bc = bass.AP(
                tensor=rz_ap.tensor,
                offset=rz_ap.offset,
                ap=[rz_ap.ap[0], [0, NG], [1, d]],
            )
            for g in range(ST // NG):
                aw = aw_all[:, g % 4]
                nc.vector.tensor_mul(
                    out=aw, in0=expb[:, g * NG:(g + 1) * NG, :], in1=rz_bc
                )
                dma_eng = nc.sync if g % 2 == 0 else nc.scalar
                dma_eng.dma_start(
                    out=attnw_r[:, g * NG:(g + 1) * NG, :], in_=aw
                )

    return nc


def _build_m2(Wq, Wk, bk):
    """[Wq^T Wk | Wq^T bk | 0-pad to a full 128-col tile] in float64."""
    d = Wq.shape[0]
    m2 = np.zeros((d, d + P), dtype=np.float64)
    m2[:, :d] = Wq.T @ Wk
    m2[:, d] = Wq.T @ bk
    return m2


def _host_prep(q, k, v, Wq, bq, Wk, bk, Wv, bv):
    """Shard over batch, fold Wq/Wk/bk into M2/wqbk, pre-transpose, cast."""
    fp8 = ml_dtypes.float8_e4m3
    bf16 = ml_dtypes.bfloat16
    q = np.asarray(q, dtype=np.float32)
    k = np.asarray(k, dtype=np.float32)
    v = np.asarray(v, dtype=np.float32)
    Wq = np.asarray(Wq, dtype=np.float64)
    Wk = np.asarray(Wk, dtype=np.float64)
    bk = np.asarray(bk, dtype=np.float64)
    m2 = _build_m2(Wq, Wk, bk).astype(fp8)       # [d(a), d(b) | alpha | 0]
    wvT = np.asarray(Wv, dtype=np.float32).T.astype(fp8)
    bv = np.ascontiguousarray(np.asarray(bv, dtype=np.float32))

    in_maps = []
    for i in range(B):
        in_maps.append(
            {
                "qT": q[i].T.astype(fp8),
                "kT": k[i].T.astype(fp8),
                "vT": v[i].T.astype(fp8),
                "m2": m2,
                "wvT": wvT,
                "bv": bv,
                "qres": (q[i] * float(S)).astype(bf16),
            }
        )
    return in_maps


def _host_post(attn_raw, attnw_raw):
    """Undo the 2048x psum scaling and widen to f32."""
    attn = attn_raw.astype(np.float32) * (1.0 / float(S))
    attn_w = attnw_raw.astype(np.float32)
    return attn, attn_w


_CACHED_NC = None


def kernel(q, k, v, Wq, bq, Wk, bk, Wv, bv):
    global _CACHED_NC
    from concourse import bass_utils

    in_maps = _host_prep(q, k, v, Wq, bq, Wk, bk, Wv, bv)
    if _CACHED_NC is None:
        _CACHED_NC = build_nc()
        _CACHED_NC.finalize()  # bacc passes (reg alloc, wait splitting)
    res = bass_utils.run_bass_kernel_spmd(
        _CACHED_NC, in_maps, core_ids=list(range(B))
    )
    attn = np.stack([np.asarray(res.results[i]["attn"]) for i in range(B)])
    attn_w = np.stack([np.asarray(res.results[i]["attn_w"]) for i in range(B)])
    return _host_post(attn, attn_w)
